# revision 4
# baseline (speedup 1.0000x reference)
"""Trainium2 Bass kernel for the ExpressionEncoder module.

Strategy (pure data parallel, 8 NeuronCores, 8 batches per core):
  - Host precomputes index-derived small tensors: the per-token scaled
    segment one-hot (folding the 1/count mean divide and the leaf-slot
    permutation), leaf gates, depth-embedding biases, merge gates and the
    topology-hash gather of shape_embed rows.
  - Device streams the big `states` tensor (64 MB/core) once from HBM and
    reduces it with PE matmuls against the one-hot (segment mean pooling),
    then runs the segment MLP, leaf scatter, bottom-up tree merges and the
    final output assembly on-chip.
All float compute is f32.
"""

import os

import numpy as np

import concourse.bass as bass
import concourse.tile as tile
from concourse import mybir
from concourse.bass_utils import run_bass_kernel_spmd
from concourse.vector_clock import ScopedClock
import bass_rust

F32 = mybir.dt.float32
N_CORES = 8

LAST_EXEC_NS = None

# ----------------------------------------------------------------------------
# Workarounds: this walrus build accepts only 1 sync-wait per instruction.
# Move excess waits onto injected same-engine nops placed just before the
# offending instruction (same engine stream -> same order -> same semantics).
# ----------------------------------------------------------------------------


def _fixup_sync_waits(nc, limit=1):
    k = 0
    for f in nc.m.functions:
        for bb in f.blocks:
            insts = list(bb.instructions)
            out, changed = [], False
            for inst in insts:
                si = inst.sync_info
                waits = list(si.on_wait) if si is not None and si.on_wait else []
                if len(waits) > limit:
                    si.on_wait = waits[-limit:]
                    for w in waits[:-limit]:
                        nop = mybir.InstNoOp(name=f"nopw-{k}", ins=[], outs=[])
                        k += 1
                        nop.engine = inst.engine
                        nop.sync_info = bass_rust.SyncInfo(on_wait=[w], on_update=[])
                        out.append(nop)
                    changed = True
                out.append(inst)
            if changed:
                bb.instructions = out


def _patched_drain_and_barrier(self, tick_clock, wait_clock):
    nc = self.nc
    drain_inst = nc.sync.drain()
    wait_clock.add_sem_waits(
        drain_inst.ins, ScopedClock({None: tick_clock.global_clock})
    )
    nc.all_engine_barrier()
    popped = nc._tile_sem_poison_stack.pop()
    assert popped is self._sem_poison
    nc.clear_and_free_semaphores(list(self.sems.allocated().values()))
    nc.all_engine_barrier()


tile.TileContext._drain_and_barrier = _patched_drain_and_barrier


# ----------------------------------------------------------------------------
# Device program
# ----------------------------------------------------------------------------

_BUILD_CACHE = {}


def _build_program(BL, SEQ, HID, OUT, S, NN, levels):
    """levels: tuple of (node_list, lc_list, rc_list) processed in order;
    every listed node gets node_reprT[col] = (merge + mbias) * gate."""
    key = (BL, SEQ, HID, OUT, S, NN, levels)
    if key in _BUILD_CACHE:
        return _BUILD_CACHE[key]

    KH = HID // 128   # k-tiles of HID (8)
    KO = OUT // 128   # o-chunks of OUT (8)
    KM = 2 * OUT // 128  # k-tiles of the merge concat (16)
    NT = SEQ // 128   # token tiles per sequence (16)
    G = 2             # token tiles per streamed chunk (1 MB DMAs)
    NC = NT // G      # chunks per sequence (8)
    R = BL * S        # enc rows per core (32)
    n_levels = len(levels)

    nc = bass.Bass("TRN2", target_bir_lowering=False)
    d_states = nc.dram_tensor("states", [BL, SEQ, HID], F32, kind="ExternalInput").ap()
    d_oh = nc.dram_tensor("oh", [128, BL, NT, S], F32, kind="ExternalInput").ap()
    d_w1 = nc.dram_tensor("w1", [128, KH, OUT], F32, kind="ExternalInput").ap()
    d_w2 = nc.dram_tensor("w2", [128, KO, OUT], F32, kind="ExternalInput").ap()
    d_wm1 = nc.dram_tensor("wm1", [128, KM, OUT], F32, kind="ExternalInput").ap()
    d_wm2 = nc.dram_tensor("wm2", [128, KO, OUT], F32, kind="ExternalInput").ap()
    d_b1 = nc.dram_tensor("b1c", [128, KO], F32, kind="ExternalInput").ap()
    d_b2 = nc.dram_tensor("b2c", [128, KO], F32, kind="ExternalInput").ap()
    d_bm1 = nc.dram_tensor("bm1c", [128, KO], F32, kind="ExternalInput").ap()
    d_mbias = nc.dram_tensor("mbias", [128, KO, max(n_levels, 1)], F32,
                             kind="ExternalInput").ap()
    d_gate = nc.dram_tensor("gate_rep", [128, BL, S], F32, kind="ExternalInput").ap()
    d_dbias = nc.dram_tensor("dbias_ok", [128, KO, BL, S], F32,
                             kind="ExternalInput").ap()
    d_mgate = nc.dram_tensor("mgate", [128, max(sum(len(lv[0]) for lv in levels), 1),
                                       BL], F32, kind="ExternalInput").ap()
    d_ident = nc.dram_tensor("ident", [128, 128], F32, kind="ExternalInput").ap()
    d_c2 = nc.dram_tensor("c2", [BL, OUT], F32, kind="ExternalInput").ap()
    d_out = nc.dram_tensor("out", [BL, OUT], F32, kind="ExternalOutput").ap()

    GELU = mybir.ActivationFunctionType.Gelu
    ADD = mybir.AluOpType.add
    MULT = mybir.AluOpType.mult

    with tile.TileContext(nc) as tc:
        with tc.tile_pool(name="consts", bufs=1) as consts, \
             tc.tile_pool(name="acts", bufs=1) as acts:
            # ---- resident constants (issued on ACT's HWDGE ring so they
            # don't sit behind the stalled states stream on SP's ring) ----
            w1 = consts.tile([128, KH, OUT], F32)
            nc.scalar.dma_start(w1[:], d_w1)
            w2 = consts.tile([128, KO, OUT], F32)
            nc.scalar.dma_start(w2[:], d_w2)
            wm1 = consts.tile([128, KM, OUT], F32)
            nc.scalar.dma_start(wm1[:], d_wm1)
            wm2 = consts.tile([128, KO, OUT], F32)
            nc.scalar.dma_start(wm2[:], d_wm2)
            oh = consts.tile([128, BL, NT, S], F32)
            nc.scalar.dma_start(oh[:], d_oh)
            b1c = consts.tile([128, KO], F32)
            nc.scalar.dma_start(b1c[:], d_b1)
            b2c = consts.tile([128, KO], F32)
            nc.scalar.dma_start(b2c[:], d_b2)
            bm1c = consts.tile([128, KO], F32)
            nc.scalar.dma_start(bm1c[:], d_bm1)
            mbias = consts.tile([128, KO, max(n_levels, 1)], F32)
            nc.scalar.dma_start(mbias[:], d_mbias)
            gate_rep = consts.tile([128, BL, S], F32)
            nc.scalar.dma_start(gate_rep[:], d_gate)
            dbias_ok = consts.tile([128, KO, BL, S], F32)
            nc.scalar.dma_start(dbias_ok[:], d_dbias)
            mgate = consts.tile([128, max(sum(len(lv[0]) for lv in levels), 1), BL],
                                F32)
            nc.scalar.dma_start(mgate[:], d_mgate)
            ident = consts.tile([128, 128], F32)
            nc.scalar.dma_start(ident[:], d_ident)
            c2sb = consts.tile([BL, OUT], F32)
            nc.scalar.dma_start(c2sb[:], d_c2)

            pooledT = acts.tile([128, KH, R], F32)   # (feat_k, (b, c))

            # ---- phase A: stream states, segment-mean pool, transpose ----
            with tc.tile_pool(name="stream", bufs=2) as stream, \
                 tc.tile_pool(name="pscr", bufs=2) as pscr, \
                 tc.tile_pool(name="pp", bufs=2, space="PSUM") as ppool, \
                 tc.tile_pool(name="tp", bufs=4, space="PSUM") as tpool:
                for b in range(BL):
                    pp = ppool.tile([S, HID], F32)
                    for ci in range(NC):
                        st = stream.tile([128, G, HID], F32)
                        nc.sync.dma_start(
                            st[:],
                            d_states[b, ci * G * 128:(ci + 1) * G * 128, :]
                            .rearrange("(g p) h -> p g h", p=128))
                        for g2 in range(G):
                            g = ci * G + g2
                            for h in range(HID // 512):
                                nc.tensor.matmul(
                                    pp[:, h * 512:(h + 1) * 512],
                                    oh[:, b, g, :],
                                    st[:, g2, h * 512:(h + 1) * 512],
                                    start=(g == 0), stop=(g == NT - 1),
                                    skip_group_check=True)
                    pooled_sb = pscr.tile([S, HID], F32)
                    nc.vector.tensor_copy(pooled_sb[:], pp[:])
                    for kc in range(KH):
                        tp = tpool.tile([128, S], F32)
                        nc.tensor.transpose(
                            tp[:], pooled_sb[:, kc * 128:(kc + 1) * 128],
                            ident[0:S, 0:S])
                        nc.vector.tensor_copy(
                            pooledT[:, kc, S * b:S * (b + 1)], tp[:])

            # ---- phase B: segment MLP -> contrib -> leaf scatter ----
            enc1T = acts.tile([128, KO, R], F32)
            contribT = acts.tile([128, KO, BL, S], F32)
            node_reprT = acts.tile([128, KO, BL, NN], F32)
            with tc.tile_pool(name="escr", bufs=2) as escr, \
                 tc.tile_pool(name="ep", bufs=4, space="PSUM") as epool:
                for oc in range(KO):
                    ps1 = epool.tile([128, R], F32)
                    for kc in range(KH):
                        nc.tensor.matmul(ps1[:],
                                         w1[:, kc, oc * 128:(oc + 1) * 128],
                                         pooledT[:, kc, :],
                                         start=(kc == 0), stop=(kc == KH - 1))
                    nc.scalar.activation(enc1T[:, oc, :], ps1[:], GELU,
                                         bias=b1c[:, oc:oc + 1], scale=1.0)
                for oc in range(KO):
                    ps2 = epool.tile([128, R], F32)
                    for kc in range(KO):
                        nc.tensor.matmul(ps2[:],
                                         w2[:, kc, oc * 128:(oc + 1) * 128],
                                         enc1T[:, kc, :],
                                         start=(kc == 0), stop=(kc == KO - 1))
                    tmp = escr.tile([128, BL, S], F32)
                    nc.vector.scalar_tensor_tensor(tmp[:], ps2[:],
                                                   b2c[:, oc:oc + 1],
                                                   gate_rep[:], ADD, MULT)
                    nc.vector.tensor_add(contribT[:, oc, :, :], tmp[:],
                                         dbias_ok[:, oc, :, :])
                nc.vector.memset(node_reprT[:], 0.0)
                for oc in range(KO):
                    nc.vector.tensor_copy(node_reprT[:, oc, :, NN - S:NN],
                                          contribT[:, oc, :, :])

            # ---- phase C: bottom-up tree merges (level-batched) ----
            gcol = 0
            with tc.tile_pool(name="mscr", bufs=2) as mscr, \
                 tc.tile_pool(name="mp", bufs=2, space="PSUM") as mpool:
                for li, (nodes, lcs, rcs) in enumerate(levels):
                    W = len(nodes)          # sibling nodes merged together
                    cols = BL * W
                    lstep = 1 if W == 1 else lcs[1] - lcs[0]
                    rstep = 1 if W == 1 else rcs[1] - rcs[0]
                    g1T = mscr.tile([128, KO, cols], F32, tag="g1T")
                    for oc in range(KO):
                        psg = mpool.tile([128, cols], F32, tag="psg")
                        for k in range(KM):
                            if k < KH:
                                rhs = node_reprT[:, k, :,
                                                 lcs[0]:lcs[0] + (W - 1) * lstep + 1:lstep]
                            else:
                                rhs = node_reprT[:, k - KH, :,
                                                 rcs[0]:rcs[0] + (W - 1) * rstep + 1:rstep]
                            nc.tensor.matmul(psg[:],
                                             wm1[:, k, oc * 128:(oc + 1) * 128],
                                             rhs, start=(k == 0),
                                             stop=(k == KM - 1))
                        nc.scalar.activation(g1T[:, oc, :], psg[:], GELU,
                                             bias=bm1c[:, oc:oc + 1], scale=1.0)
                    for oc in range(KO):
                        psm = mpool.tile([128, cols], F32, tag="psm")
                        for k in range(KO):
                            nc.tensor.matmul(psm[:],
                                             wm2[:, k, oc * 128:(oc + 1) * 128],
                                             g1T[:, k, :],
                                             start=(k == 0), stop=(k == KO - 1))
                        n0 = nodes[0]
                        nstep = 1 if W == 1 else nodes[1] - nodes[0]
                        outap = node_reprT[:, oc, :,
                                           n0:n0 + (W - 1) * nstep + 1:nstep]
                        nc.vector.scalar_tensor_tensor(
                            outap, psm[:].rearrange("p (b w) -> p b w", w=W),
                            mbias[:, oc, li:li + 1],
                            mgate[:, gcol:gcol + W, :]
                            .rearrange("p w b -> p b w"),
                            ADD, MULT)
                    gcol += W

                # ---- output: transpose root col, add shape-hash rows ----
                out_sb = acts.tile([BL, OUT], F32)
                for kc in range(KO):
                    tout = mpool.tile([BL, 128], F32, tag="tout")
                    nc.tensor.transpose(tout[:], node_reprT[:, kc, :, 0],
                                        ident[:, :])
                    nc.vector.tensor_add(out_sb[:, kc * 128:(kc + 1) * 128],
                                         tout[:], c2sb[:, kc * 128:(kc + 1) * 128])
                nc.sync.dma_start(d_out, out_sb[:])

    _fixup_sync_waits(nc)
    _BUILD_CACHE[key] = nc
    return nc


# ----------------------------------------------------------------------------
# Host side
# ----------------------------------------------------------------------------


def kernel(states, mask, lengths, boundaries, leaf_order, active, is_leaf,
           left_child, right_child, depth,
           W1, b1, W2, b2, Wm1, bm1, Wm2, bm2, depth_embed, shape_embed):
    global LAST_EXEC_NS
    states = np.ascontiguousarray(np.asarray(states, dtype=np.float32))
    mask = np.asarray(mask).astype(bool)
    lengths = np.asarray(lengths).astype(np.int64)
    boundaries = np.asarray(boundaries).astype(np.int64)
    leaf_order = np.asarray(leaf_order).astype(np.int64)
    active = np.asarray(active).astype(bool)
    is_leaf = np.asarray(is_leaf).astype(bool)
    lc = np.asarray(left_child).astype(np.int64)
    rc = np.asarray(right_child).astype(np.int64)
    depth = np.asarray(depth).astype(np.int64)
    W1 = np.asarray(W1, dtype=np.float32)
    b1 = np.asarray(b1, dtype=np.float32)
    W2 = np.asarray(W2, dtype=np.float32)
    b2 = np.asarray(b2, dtype=np.float32)
    Wm1 = np.asarray(Wm1, dtype=np.float32)
    bm1 = np.asarray(bm1, dtype=np.float32)
    Wm2 = np.asarray(Wm2, dtype=np.float32)
    bm2 = np.asarray(bm2, dtype=np.float32)
    depth_embed = np.asarray(depth_embed, dtype=np.float32)
    shape_embed = np.asarray(shape_embed, dtype=np.float32)

    B, SEQ, HID = states.shape
    OUT = W1.shape[1]
    S = boundaries.shape[1]
    NN = lc.shape[0]
    assert B % N_CORES == 0
    BL = B // N_CORES

    # ---- segment structure -> scaled one-hot in leaf-column space ----
    pos = np.arange(SEQ)
    seg_id = (pos[None, None, :] >= boundaries[:, :, None]).sum(1) - 1  # (B,SEQ)
    valid = mask & (pos[None, :] < lengths[:, None]) & (seg_id >= 0)
    oh_raw = (seg_id[:, :, None] == np.arange(S)[None, None, :]) & valid[:, :, None]
    oh_raw = oh_raw.astype(np.float32)                                  # (B,SEQ,S)
    cnt = oh_raw.sum(1)                                                 # (B,S)
    scale = 1.0 / np.clip(cnt, 1.0, None)

    nidx = leaf_order                                                   # (B,S)
    nclip = np.maximum(nidx, 0)
    leaf_ok = ((nidx >= 0) & np.take_along_axis(is_leaf, nclip, axis=1)
               & (cnt > 0)).astype(np.float32)                          # (B,S)
    dbias = depth_embed[depth[nclip]]                                   # (B,S,OUT)

    # leaf slots are the last S nodes; map each segment to its slot column.
    assert (leaf_order == leaf_order[0]).all(), "batch-varying leaf_order"
    lo = leaf_order[0]
    seg_of_col = {}
    for s in range(S):
        n = int(lo[s])
        if NN - S <= n < NN:
            c = n - (NN - S)
            assert c not in seg_of_col, "duplicate leaf target"
            seg_of_col[c] = s
        else:
            assert not leaf_ok[:, s].any(), "contrib to non-leaf-slot node"

    ohc = np.zeros((B, SEQ, S), np.float32)
    gate_col = np.zeros((B, S), np.float32)
    dbias_ok_col = np.zeros((B, S, OUT), np.float32)
    for c, s in seg_of_col.items():
        ohc[:, :, c] = oh_raw[:, :, s] * scale[:, s, None]
        gate_col[:, c] = leaf_ok[:, s]
        dbias_ok_col[:, c, :] = dbias[:, s, :] * leaf_ok[:, s, None]

    # ---- merge schedule: reference processes i = NN-1 .. 0 ----
    is_int = (active & ~is_leaf)                                        # (B,NN)
    merge_nodes = []
    for i in range(NN - 1, -1, -1):
        if lc[i] != -1 and bool(is_int[:, i].any()):
            merge_nodes.append(i)
    # group into levels of sibling nodes with uniform child/col strides
    levels = []
    for i in merge_nodes:
        placed = False
        if levels:
            nodes, lcs, rcs = levels[-1]
            if depth[i] == depth[nodes[0]] and i < nodes[0]:
                cand_n = [i] + nodes
                cand_l = [int(lc[i])] + lcs
                cand_r = [int(rc[i])] + rcs
                def _uniform(xs):
                    return len(xs) < 2 or len({xs[j + 1] - xs[j]
                                               for j in range(len(xs) - 1)}) == 1
                if (_uniform(cand_n) and _uniform(cand_l) and _uniform(cand_r)
                        and (len(cand_l) < 2 or cand_l[1] > cand_l[0])
                        and (len(cand_r) < 2 or cand_r[1] > cand_r[0])
                        and (len(cand_n) < 2 or cand_n[1] > cand_n[0])):
                    levels[-1] = (cand_n, cand_l, cand_r)
                    placed = True
        if not placed:
            levels.append(([i], [int(lc[i])], [int(rc[i])]))
    levels_key = tuple((tuple(n), tuple(l), tuple(r)) for n, l, r in levels)

    # ---- topology hash -> shape embedding rows ----
    pattern = active.astype(np.int64) * 2 + is_leaf.astype(np.int64)
    weights = 31 ** np.arange(NN, dtype=np.int64)
    hashed = (pattern * weights[None, :]).sum(1)
    shape_ids = np.abs(hashed) % shape_embed.shape[0]
    c2 = np.ascontiguousarray(shape_embed[shape_ids])                   # (B,OUT)

    # ---- device-layout constant tensors (shared across cores) ----
    KH, KO, KM = HID // 128, OUT // 128, 2 * OUT // 128
    NT = SEQ // 128
    W1r = np.ascontiguousarray(W1.reshape(KH, 128, OUT).transpose(1, 0, 2))
    W2r = np.ascontiguousarray(W2.reshape(KO, 128, OUT).transpose(1, 0, 2))
    Wm1r = np.ascontiguousarray(Wm1.reshape(KM, 128, OUT).transpose(1, 0, 2))
    Wm2r = np.ascontiguousarray(Wm2.reshape(KO, 128, OUT).transpose(1, 0, 2))
    b1r = np.ascontiguousarray(b1.reshape(KO, 128).T)
    b2r = np.ascontiguousarray(b2.reshape(KO, 128).T)
    bm1r = np.ascontiguousarray(bm1.reshape(KO, 128).T)
    n_levels = len(levels)
    mbias = np.zeros((128, KO, max(n_levels, 1)), np.float32)
    for li, (nodes, _, _) in enumerate(levels):
        v = bm2 + depth_embed[depth[nodes[0]]]
        mbias[:, :, li] = v.reshape(KO, 128).T
    ident = np.eye(128, dtype=np.float32)
    total_gcols = max(sum(len(lv[0]) for lv in levels), 1)

    nc = _build_program(BL, SEQ, HID, OUT, S, NN, levels_key)

    in_maps = []
    for ci in range(N_CORES):
        bs = slice(ci * BL, (ci + 1) * BL)
        ohT = np.ascontiguousarray(
            ohc[bs].reshape(BL, NT, 128, S).transpose(2, 0, 1, 3))
        gate_rep = np.ascontiguousarray(
            np.broadcast_to(gate_col[bs][None], (128, BL, S)))
        dbT = np.ascontiguousarray(
            dbias_ok_col[bs].transpose(2, 0, 1).reshape(KO, 128, BL, S)
            .transpose(1, 0, 2, 3))
        mg = np.zeros((128, total_gcols, BL), np.float32)
        gcol = 0
        for (nodes, _, _) in levels:
            for w, n in enumerate(nodes):
                mg[:, gcol + w, :] = is_int[bs, n].astype(np.float32)[None, :]
            gcol += len(nodes)
        in_maps.append({
            "states": states[bs],
            "oh": ohT,
            "w1": W1r, "w2": W2r, "wm1": Wm1r, "wm2": Wm2r,
            "b1c": b1r, "b2c": b2r, "bm1c": bm1r,
            "mbias": mbias,
            "gate_rep": gate_rep,
            "dbias_ok": dbT,
            "mgate": mg,
            "ident": ident,
            "c2": np.ascontiguousarray(c2[bs]),
        })

    trace = bool(int(os.environ.get("KERNEL_BASS_TRACE", "0")))
    res = run_bass_kernel_spmd(nc, in_maps, list(range(N_CORES)), trace=trace)
    LAST_EXEC_NS = res.exec_time_ns
    out = np.concatenate([res.results[i]["out"] for i in range(N_CORES)], axis=0)
    return out.astype(np.float32)


# revision 6
# speedup vs baseline: 2.4994x; 2.4994x over previous
"""Trainium2 Bass kernel for the ExpressionEncoder module.

Strategy (pure data parallel, 8 NeuronCores, 8 batches per core):
  - Host precomputes index-derived small tensors: the per-token segment
    one-hot (exact 0/1, bf16), per-segment 1/count scales, leaf gates,
    depth-embedding biases, merge gates and the topology-hash gather of
    shape_embed rows.
  - Device streams the big `states` tensor (64 MB f32/core) once from HBM
    with an on-the-fly cast to bf16 (SWDGE), reduces it with PE matmuls
    against the one-hot (segment sums, f32 PSUM accumulation), applies the
    exact f32 1/count scale, then runs the segment MLP, leaf scatter,
    bottom-up tree merges and the final output assembly on-chip in bf16
    with f32 accumulation.
"""

import os

import numpy as np
import ml_dtypes

import concourse.bass as bass
import concourse.tile as tile
from concourse import mybir
from concourse.bass_utils import run_bass_kernel_spmd
from concourse.vector_clock import ScopedClock
import bass_rust

F32 = mybir.dt.float32
BF16 = mybir.dt.bfloat16
NP_BF16 = np.dtype(ml_dtypes.bfloat16)
N_CORES = 8

LAST_EXEC_NS = None

# ----------------------------------------------------------------------------
# Workarounds: this walrus build accepts only 1 sync-wait per instruction.
# Move excess waits onto injected same-engine nops placed just before the
# offending instruction (same engine stream -> same order -> same semantics).
# ----------------------------------------------------------------------------


def _fixup_sync_waits(nc, limit=1):
    k = 0
    for f in nc.m.functions:
        for bb in f.blocks:
            insts = list(bb.instructions)
            out, changed = [], False
            for inst in insts:
                si = inst.sync_info
                waits = list(si.on_wait) if si is not None and si.on_wait else []
                if len(waits) > limit:
                    si.on_wait = waits[-limit:]
                    for w in waits[:-limit]:
                        nop = mybir.InstNoOp(name=f"nopw-{k}", ins=[], outs=[])
                        k += 1
                        nop.engine = inst.engine
                        nop.sync_info = bass_rust.SyncInfo(on_wait=[w], on_update=[])
                        out.append(nop)
                    changed = True
                out.append(inst)
            if changed:
                bb.instructions = out


def _patched_drain_and_barrier(self, tick_clock, wait_clock):
    nc = self.nc
    drain_inst = nc.sync.drain()
    wait_clock.add_sem_waits(
        drain_inst.ins, ScopedClock({None: tick_clock.global_clock})
    )
    nc.all_engine_barrier()
    popped = nc._tile_sem_poison_stack.pop()
    assert popped is self._sem_poison
    nc.clear_and_free_semaphores(list(self.sems.allocated().values()))
    nc.all_engine_barrier()


tile.TileContext._drain_and_barrier = _patched_drain_and_barrier


# ----------------------------------------------------------------------------
# Device program
# ----------------------------------------------------------------------------

_BUILD_CACHE = {}


def _build_program(BL, SEQ, HID, OUT, S, NN, levels):
    """levels: tuple of (node_list, lc_list, rc_list) processed in order;
    every listed node gets node_reprT[col] = (merge + mbias) * gate."""
    key = (BL, SEQ, HID, OUT, S, NN, levels)
    if key in _BUILD_CACHE:
        return _BUILD_CACHE[key]

    KH = HID // 128      # k-tiles of HID (8)
    KO = OUT // 128      # o-chunks of OUT (8)
    KM = 2 * OUT // 128  # k-tiles of the merge concat (16)
    NT = SEQ // 128      # token tiles per sequence (16)
    G = 4                # token tiles per streamed chunk (2 MB f32 reads)
    NC = NT // G         # chunks per sequence (4)
    R = BL * S           # enc rows per core (32)
    n_levels = len(levels)
    total_gcols = max(sum(len(lv[0]) for lv in levels), 1)

    nc = bass.Bass("TRN2", target_bir_lowering=False)
    d_states = nc.dram_tensor("states", [BL, SEQ, HID], F32, kind="ExternalInput").ap()
    d_oh = nc.dram_tensor("oh", [128, BL, NT, S], BF16, kind="ExternalInput").ap()
    d_w1 = nc.dram_tensor("w1", [128, KH, OUT], BF16, kind="ExternalInput").ap()
    d_w2 = nc.dram_tensor("w2", [128, KO, OUT], BF16, kind="ExternalInput").ap()
    d_wm1 = nc.dram_tensor("wm1", [128, KM, OUT], BF16, kind="ExternalInput").ap()
    d_wm2 = nc.dram_tensor("wm2", [128, KO, OUT], BF16, kind="ExternalInput").ap()
    d_b1 = nc.dram_tensor("b1c", [128, KO], F32, kind="ExternalInput").ap()
    d_b2 = nc.dram_tensor("b2c", [128, KO], F32, kind="ExternalInput").ap()
    d_bm1 = nc.dram_tensor("bm1c", [128, KO], F32, kind="ExternalInput").ap()
    d_mbias = nc.dram_tensor("mbias", [128, KO, max(n_levels, 1)], F32,
                             kind="ExternalInput").ap()
    d_gate = nc.dram_tensor("gate_rep", [128, BL, S], F32, kind="ExternalInput").ap()
    d_dbias = nc.dram_tensor("dbias_ok", [128, KO, BL, S], F32,
                             kind="ExternalInput").ap()
    d_mgate = nc.dram_tensor("mgate", [128, total_gcols, BL], F32,
                             kind="ExternalInput").ap()
    d_recip = nc.dram_tensor("recip", [S, BL], F32, kind="ExternalInput").ap()
    d_ident = nc.dram_tensor("ident", [128, 128], BF16, kind="ExternalInput").ap()
    d_c2 = nc.dram_tensor("c2", [BL, OUT], F32, kind="ExternalInput").ap()
    d_out = nc.dram_tensor("out", [BL, OUT], F32, kind="ExternalOutput").ap()

    GELU = mybir.ActivationFunctionType.Gelu
    ADD = mybir.AluOpType.add
    MULT = mybir.AluOpType.mult

    with tile.TileContext(nc) as tc:
        with tc.tile_pool(name="consts", bufs=1) as consts, \
             tc.tile_pool(name="acts", bufs=1) as acts:
            # ---- resident constants (issued on ACT's HWDGE ring so they
            # don't sit behind the states stream on SP's ring) ----
            w1 = consts.tile([128, KH, OUT], BF16)
            nc.scalar.dma_start(w1[:], d_w1)
            w2 = consts.tile([128, KO, OUT], BF16)
            nc.scalar.dma_start(w2[:], d_w2)
            wm1 = consts.tile([128, KM, OUT], BF16)
            nc.scalar.dma_start(wm1[:], d_wm1)
            wm2 = consts.tile([128, KO, OUT], BF16)
            nc.scalar.dma_start(wm2[:], d_wm2)
            oh = consts.tile([128, BL, NT, S], BF16)
            nc.scalar.dma_start(oh[:], d_oh)
            b1c = consts.tile([128, KO], F32)
            nc.scalar.dma_start(b1c[:], d_b1)
            b2c = consts.tile([128, KO], F32)
            nc.scalar.dma_start(b2c[:], d_b2)
            bm1c = consts.tile([128, KO], F32)
            nc.scalar.dma_start(bm1c[:], d_bm1)
            mbias = consts.tile([128, KO, max(n_levels, 1)], F32)
            nc.scalar.dma_start(mbias[:], d_mbias)
            gate_rep = consts.tile([128, BL, S], F32)
            nc.scalar.dma_start(gate_rep[:], d_gate)
            dbias_ok = consts.tile([128, KO, BL, S], F32)
            nc.scalar.dma_start(dbias_ok[:], d_dbias)
            mgate = consts.tile([128, total_gcols, BL], F32)
            nc.scalar.dma_start(mgate[:], d_mgate)
            recip = consts.tile([S, BL], F32)
            nc.scalar.dma_start(recip[:], d_recip)
            ident = consts.tile([128, 128], BF16)
            nc.scalar.dma_start(ident[:], d_ident)
            c2sb = consts.tile([BL, OUT], F32)
            nc.scalar.dma_start(c2sb[:], d_c2)

            pooledT = acts.tile([128, KH, R], BF16)   # (feat_k, (b, c))

            # ---- phase A: stream states (cast to bf16), segment-sum pool,
            # exact f32 1/count scale, transpose ----
            with tc.tile_pool(name="stream", bufs=4) as stream, \
                 tc.tile_pool(name="pscr", bufs=2) as pscr, \
                 tc.tile_pool(name="pp", bufs=2, space="PSUM") as ppool, \
                 tc.tile_pool(name="tp", bufs=4, space="PSUM") as tpool:
                for b in range(BL):
                    pp = ppool.tile([S, HID], F32)
                    for ci in range(NC):
                        st = stream.tile([128, G, HID], BF16)
                        nc.gpsimd.dma_start(
                            st[:],
                            d_states[b, ci * G * 128:(ci + 1) * G * 128, :]
                            .rearrange("(g p) h -> p g h", p=128))
                        for g2 in range(G):
                            g = ci * G + g2
                            for h in range(HID // 512):
                                nc.tensor.matmul(
                                    pp[:, h * 512:(h + 1) * 512],
                                    oh[:, b, g, :],
                                    st[:, g2, h * 512:(h + 1) * 512],
                                    start=(g == 0), stop=(g == NT - 1),
                                    skip_group_check=True)
                    pooled_sb = pscr.tile([S, HID], BF16)
                    nc.vector.tensor_scalar_mul(pooled_sb[:], pp[:],
                                                recip[:, b:b + 1])
                    for kc in range(KH):
                        tp = tpool.tile([128, S], BF16)
                        nc.tensor.transpose(
                            tp[:], pooled_sb[:, kc * 128:(kc + 1) * 128],
                            ident[0:S, 0:S])
                        nc.vector.tensor_copy(
                            pooledT[:, kc, S * b:S * (b + 1)], tp[:])

            # ---- phase B: segment MLP -> contrib -> leaf scatter ----
            enc1T = acts.tile([128, KO, R], BF16)
            contribT = acts.tile([128, KO, BL, S], BF16)
            node_reprT = acts.tile([128, KO, BL, NN], BF16)
            with tc.tile_pool(name="escr", bufs=2) as escr, \
                 tc.tile_pool(name="ep", bufs=4, space="PSUM") as epool:
                for oc in range(KO):
                    ps1 = epool.tile([128, R], F32)
                    for kc in range(KH):
                        nc.tensor.matmul(ps1[:],
                                         w1[:, kc, oc * 128:(oc + 1) * 128],
                                         pooledT[:, kc, :],
                                         start=(kc == 0), stop=(kc == KH - 1))
                    nc.scalar.activation(enc1T[:, oc, :], ps1[:], GELU,
                                         bias=b1c[:, oc:oc + 1], scale=1.0)
                for oc in range(KO):
                    ps2 = epool.tile([128, R], F32)
                    for kc in range(KO):
                        nc.tensor.matmul(ps2[:],
                                         w2[:, kc, oc * 128:(oc + 1) * 128],
                                         enc1T[:, kc, :],
                                         start=(kc == 0), stop=(kc == KO - 1))
                    tmp = escr.tile([128, BL, S], F32)
                    nc.vector.scalar_tensor_tensor(tmp[:], ps2[:],
                                                   b2c[:, oc:oc + 1],
                                                   gate_rep[:], ADD, MULT)
                    nc.vector.tensor_add(contribT[:, oc, :, :], tmp[:],
                                         dbias_ok[:, oc, :, :])
                nc.vector.memset(node_reprT[:], 0.0)
                for oc in range(KO):
                    nc.vector.tensor_copy(node_reprT[:, oc, :, NN - S:NN],
                                          contribT[:, oc, :, :])

            # ---- phase C: bottom-up tree merges (level-batched) ----
            gcol = 0
            with tc.tile_pool(name="mscr", bufs=2) as mscr, \
                 tc.tile_pool(name="mp", bufs=2, space="PSUM") as mpool:
                for li, (nodes, lcs, rcs) in enumerate(levels):
                    W = len(nodes)          # sibling nodes merged together
                    cols = BL * W
                    lstep = 1 if W == 1 else lcs[1] - lcs[0]
                    rstep = 1 if W == 1 else rcs[1] - rcs[0]
                    g1T = mscr.tile([128, KO, cols], BF16, tag="g1T")
                    for oc in range(KO):
                        psg = mpool.tile([128, cols], F32, tag="psg")
                        for k in range(KM):
                            if k < KH:
                                rhs = node_reprT[:, k, :,
                                                 lcs[0]:lcs[0] + (W - 1) * lstep + 1:lstep]
                            else:
                                rhs = node_reprT[:, k - KH, :,
                                                 rcs[0]:rcs[0] + (W - 1) * rstep + 1:rstep]
                            nc.tensor.matmul(psg[:],
                                             wm1[:, k, oc * 128:(oc + 1) * 128],
                                             rhs, start=(k == 0),
                                             stop=(k == KM - 1))
                        nc.scalar.activation(g1T[:, oc, :], psg[:], GELU,
                                             bias=bm1c[:, oc:oc + 1], scale=1.0)
                    for oc in range(KO):
                        psm = mpool.tile([128, cols], F32, tag="psm")
                        for k in range(KO):
                            nc.tensor.matmul(psm[:],
                                             wm2[:, k, oc * 128:(oc + 1) * 128],
                                             g1T[:, k, :],
                                             start=(k == 0), stop=(k == KO - 1))
                        n0 = nodes[0]
                        nstep = 1 if W == 1 else nodes[1] - nodes[0]
                        outap = node_reprT[:, oc, :,
                                           n0:n0 + (W - 1) * nstep + 1:nstep]
                        nc.vector.scalar_tensor_tensor(
                            outap, psm[:].rearrange("p (b w) -> p b w", w=W),
                            mbias[:, oc, li:li + 1],
                            mgate[:, gcol:gcol + W, :]
                            .rearrange("p w b -> p b w"),
                            ADD, MULT)
                    gcol += W

                # ---- output: transpose root col, add shape-hash rows ----
                out_sb = acts.tile([BL, OUT], F32)
                for kc in range(KO):
                    tout = mpool.tile([BL, 128], BF16, tag="tout")
                    nc.tensor.transpose(tout[:], node_reprT[:, kc, :, 0],
                                        ident[:, :])
                    nc.vector.tensor_add(out_sb[:, kc * 128:(kc + 1) * 128],
                                         tout[:], c2sb[:, kc * 128:(kc + 1) * 128])
                nc.sync.dma_start(d_out, out_sb[:])

    _fixup_sync_waits(nc)
    _BUILD_CACHE[key] = nc
    return nc


# ----------------------------------------------------------------------------
# Host side
# ----------------------------------------------------------------------------


def kernel(states, mask, lengths, boundaries, leaf_order, active, is_leaf,
           left_child, right_child, depth,
           W1, b1, W2, b2, Wm1, bm1, Wm2, bm2, depth_embed, shape_embed):
    global LAST_EXEC_NS
    states = np.ascontiguousarray(np.asarray(states, dtype=np.float32))
    mask = np.asarray(mask).astype(bool)
    lengths = np.asarray(lengths).astype(np.int64)
    boundaries = np.asarray(boundaries).astype(np.int64)
    leaf_order = np.asarray(leaf_order).astype(np.int64)
    active = np.asarray(active).astype(bool)
    is_leaf = np.asarray(is_leaf).astype(bool)
    lc = np.asarray(left_child).astype(np.int64)
    rc = np.asarray(right_child).astype(np.int64)
    depth = np.asarray(depth).astype(np.int64)
    W1 = np.asarray(W1, dtype=np.float32)
    b1 = np.asarray(b1, dtype=np.float32)
    W2 = np.asarray(W2, dtype=np.float32)
    b2 = np.asarray(b2, dtype=np.float32)
    Wm1 = np.asarray(Wm1, dtype=np.float32)
    bm1 = np.asarray(bm1, dtype=np.float32)
    Wm2 = np.asarray(Wm2, dtype=np.float32)
    bm2 = np.asarray(bm2, dtype=np.float32)
    depth_embed = np.asarray(depth_embed, dtype=np.float32)
    shape_embed = np.asarray(shape_embed, dtype=np.float32)

    B, SEQ, HID = states.shape
    OUT = W1.shape[1]
    S = boundaries.shape[1]
    NN = lc.shape[0]
    assert B % N_CORES == 0
    BL = B // N_CORES

    # ---- segment structure -> one-hot in leaf-column space ----
    pos = np.arange(SEQ)
    seg_id = (pos[None, None, :] >= boundaries[:, :, None]).sum(1) - 1  # (B,SEQ)
    valid = mask & (pos[None, :] < lengths[:, None]) & (seg_id >= 0)
    oh_raw = (seg_id[:, :, None] == np.arange(S)[None, None, :]) & valid[:, :, None]
    oh_raw = oh_raw.astype(np.float32)                                  # (B,SEQ,S)
    cnt = oh_raw.sum(1)                                                 # (B,S)
    scale = 1.0 / np.clip(cnt, 1.0, None)

    nidx = leaf_order                                                   # (B,S)
    nclip = np.maximum(nidx, 0)
    leaf_ok = ((nidx >= 0) & np.take_along_axis(is_leaf, nclip, axis=1)
               & (cnt > 0)).astype(np.float32)                          # (B,S)
    dbias = depth_embed[depth[nclip]]                                   # (B,S,OUT)

    # leaf slots are the last S nodes; map each segment to its slot column.
    assert (leaf_order == leaf_order[0]).all(), "batch-varying leaf_order"
    lo = leaf_order[0]
    seg_of_col = {}
    for s in range(S):
        n = int(lo[s])
        if NN - S <= n < NN:
            c = n - (NN - S)
            assert c not in seg_of_col, "duplicate leaf target"
            seg_of_col[c] = s
        else:
            assert not leaf_ok[:, s].any(), "contrib to non-leaf-slot node"

    ohc = np.zeros((B, SEQ, S), np.float32)
    recip_col = np.ones((B, S), np.float32)
    gate_col = np.zeros((B, S), np.float32)
    dbias_ok_col = np.zeros((B, S, OUT), np.float32)
    for c, s in seg_of_col.items():
        ohc[:, :, c] = oh_raw[:, :, s]
        recip_col[:, c] = scale[:, s]
        gate_col[:, c] = leaf_ok[:, s]
        dbias_ok_col[:, c, :] = dbias[:, s, :] * leaf_ok[:, s, None]

    # ---- merge schedule: reference processes i = NN-1 .. 0 ----
    is_int = (active & ~is_leaf)                                        # (B,NN)
    merge_nodes = []
    for i in range(NN - 1, -1, -1):
        if lc[i] != -1 and bool(is_int[:, i].any()):
            merge_nodes.append(i)
    # group into levels of sibling nodes with uniform child/col strides
    levels = []
    for i in merge_nodes:
        placed = False
        if levels:
            nodes, lcs, rcs = levels[-1]
            if depth[i] == depth[nodes[0]] and i < nodes[0]:
                cand_n = [i] + nodes
                cand_l = [int(lc[i])] + lcs
                cand_r = [int(rc[i])] + rcs

                def _uniform(xs):
                    return len(xs) < 2 or len({xs[j + 1] - xs[j]
                                               for j in range(len(xs) - 1)}) == 1
                if (_uniform(cand_n) and _uniform(cand_l) and _uniform(cand_r)
                        and (len(cand_l) < 2 or cand_l[1] > cand_l[0])
                        and (len(cand_r) < 2 or cand_r[1] > cand_r[0])
                        and (len(cand_n) < 2 or cand_n[1] > cand_n[0])):
                    levels[-1] = (cand_n, cand_l, cand_r)
                    placed = True
        if not placed:
            levels.append(([i], [int(lc[i])], [int(rc[i])]))
    levels_key = tuple((tuple(n), tuple(l), tuple(r)) for n, l, r in levels)

    # ---- topology hash -> shape embedding rows ----
    pattern = active.astype(np.int64) * 2 + is_leaf.astype(np.int64)
    weights = 31 ** np.arange(NN, dtype=np.int64)
    hashed = (pattern * weights[None, :]).sum(1)
    shape_ids = np.abs(hashed) % shape_embed.shape[0]
    c2 = np.ascontiguousarray(shape_embed[shape_ids])                   # (B,OUT)

    # ---- device-layout constant tensors (shared across cores) ----
    KH, KO, KM = HID // 128, OUT // 128, 2 * OUT // 128
    NT = SEQ // 128
    W1r = np.ascontiguousarray(
        W1.reshape(KH, 128, OUT).transpose(1, 0, 2)).astype(NP_BF16)
    W2r = np.ascontiguousarray(
        W2.reshape(KO, 128, OUT).transpose(1, 0, 2)).astype(NP_BF16)
    Wm1r = np.ascontiguousarray(
        Wm1.reshape(KM, 128, OUT).transpose(1, 0, 2)).astype(NP_BF16)
    Wm2r = np.ascontiguousarray(
        Wm2.reshape(KO, 128, OUT).transpose(1, 0, 2)).astype(NP_BF16)
    b1r = np.ascontiguousarray(b1.reshape(KO, 128).T)
    b2r = np.ascontiguousarray(b2.reshape(KO, 128).T)
    bm1r = np.ascontiguousarray(bm1.reshape(KO, 128).T)
    n_levels = len(levels)
    mbias = np.zeros((128, KO, max(n_levels, 1)), np.float32)
    for li, (nodes, _, _) in enumerate(levels):
        v = bm2 + depth_embed[depth[nodes[0]]]
        mbias[:, :, li] = v.reshape(KO, 128).T
    ident = np.eye(128).astype(NP_BF16)
    total_gcols = max(sum(len(lv[0]) for lv in levels), 1)

    nc = _build_program(BL, SEQ, HID, OUT, S, NN, levels_key)

    in_maps = []
    for ci in range(N_CORES):
        bs = slice(ci * BL, (ci + 1) * BL)
        ohT = np.ascontiguousarray(
            ohc[bs].reshape(BL, NT, 128, S).transpose(2, 0, 1, 3)).astype(NP_BF16)
        gate_rep = np.ascontiguousarray(
            np.broadcast_to(gate_col[bs][None], (128, BL, S)))
        dbT = np.ascontiguousarray(
            dbias_ok_col[bs].transpose(2, 0, 1).reshape(KO, 128, BL, S)
            .transpose(1, 0, 2, 3))
        mg = np.zeros((128, total_gcols, BL), np.float32)
        gcol = 0
        for (nodes, _, _) in levels:
            for w, n in enumerate(nodes):
                mg[:, gcol + w, :] = is_int[bs, n].astype(np.float32)[None, :]
            gcol += len(nodes)
        in_maps.append({
            "states": states[bs],
            "oh": ohT,
            "w1": W1r, "w2": W2r, "wm1": Wm1r, "wm2": Wm2r,
            "b1c": b1r, "b2c": b2r, "bm1c": bm1r,
            "mbias": mbias,
            "gate_rep": gate_rep,
            "dbias_ok": dbT,
            "mgate": mg,
            "recip": np.ascontiguousarray(recip_col[bs].T),
            "ident": ident,
            "c2": np.ascontiguousarray(c2[bs]),
        })

    trace = bool(int(os.environ.get("KERNEL_BASS_TRACE", "0")))
    res = run_bass_kernel_spmd(nc, in_maps, list(range(N_CORES)), trace=trace)
    LAST_EXEC_NS = res.exec_time_ns
    out = np.concatenate([res.results[i]["out"] for i in range(N_CORES)], axis=0)
    return out.astype(np.float32)
